# revision 8
# baseline (speedup 1.0000x reference)
"""MoE expert-parallel FFN kernel for Trainium2 (8 NeuronCores).

Problem: per-expert GEMM -> ReLU -> per-expert GEMM
  dispatched_input: (E=8, C=2048, M=2048) f32
  inner_experts:    (E=8, M=2048, H=8192) f32
  out_experts:      (E=8, H=8192, M=2048) f32
  out:              (E=8, C=2048, M=2048) f32

Sharding: pure expert parallelism — expert e runs entirely on core e.
No collectives needed.

Per-core dataflow (bf16 compute, fp32 PSUM accumulation):
  Phase 0: transpose X (C,M) -> X^T (M,C) resident in SBUF as bf16.
           Two modes: "xbar" (cast-DMA to DRAM then DMA-transpose loads,
           zero PE time) or "pe" (TensorE 128x128 transposes).
  Phase 1: actT[h,c] = relu(W1^T X^T) streamed over h, accumulated over
           m in PSUM. W1 streamed once; ReLU+cast on ScalarE; actT
           spilled to DRAM scratch as bf16 (32MB, one tile per 128-row
           h-block so phase 2 can prefetch rows as they land).
  Phase 2: Y[c,m] = actT^T @ W2 accumulated over h in PSUM. actT
           c-strips cached in SBUF (16MB as 8 subtiles; 5 live in a
           dedicated zone so strip loads overlap phase 1); W2 streamed
           twice.
"""

import numpy as np

import concourse.bass as bass
import concourse.tile as tile
from concourse import bacc, mybir
from concourse.bass_utils import run_bass_kernel_spmd
from concourse.masks import make_identity

E = 8
C = 2048  # tokens per expert
M = 2048  # model dim
H = 8192  # ffn dim
P = 128   # partitions
FD = 512  # matmul moving free dim (one PSUM bank of fp32)

BF = mybir.dt.bfloat16
F32 = mybir.dt.float32

MT = M // P   # 16 m-tiles
CT = C // P   # 16 c-tiles
HT = H // P   # 64 h-tiles

PHASE0_MODE = "xbar"   # "xbar" or "pe"

_CACHED_NC = None


def _phase0_pe(nc, tc, x, xT):
    """TensorE-transpose path: X -> SBUF fp32 -> bf16 -> PE transpose."""
    with tc.tile_pool(name="xstage", bufs=3) as xs_pool, \
         tc.tile_pool(name="xbf", bufs=3) as xb_pool, \
         tc.tile_pool(name="tpsum", bufs=8, space="PSUM") as tp_pool, \
         tc.tile_pool(name="ident", bufs=1) as id_pool:
        ident = id_pool.tile([P, P], BF)
        make_identity(nc, ident)
        for ct in range(CT):
            # halves of the row-slab DMA so casts start earlier
            xs = xs_pool.tile([P, M], F32, tag="xs", name=f"xs{ct}")
            for half in range(2):
                nc.sync.dma_start(
                    xs[:, half * (M // 2):(half + 1) * (M // 2)],
                    x[ct * P:(ct + 1) * P,
                      half * (M // 2):(half + 1) * (M // 2)])
            xb = xb_pool.tile([P, M], BF, tag="xb", name=f"xb{ct}")
            for q in range(4):
                nc.vector.tensor_copy(
                    xb[:, q * FD:(q + 1) * FD], xs[:, q * FD:(q + 1) * FD])
                for mt in range(q * 4, (q + 1) * 4):
                    tp = tp_pool.tile([P, P], BF, tag="tp", name=f"tp{ct}_{mt}")
                    nc.tensor.transpose(
                        tp[:], xb[:, mt * P:(mt + 1) * P], ident[:])
                    nc.vector.tensor_copy(
                        xT[:, mt * C + ct * P: mt * C + (ct + 1) * P], tp[:])


def _phase0_xbar(nc, tc, x, xT, dram_pool):
    """Cast-DMA X to bf16 in DRAM, then xbar DMA-transpose into SBUF."""
    xbf = dram_pool.tile([C, M], BF, name="xbf", tag="xbf")
    for ct in range(CT):
        # SWDGE cast-DMA (fp32 -> bf16), DRAM -> DRAM
        nc.gpsimd.dma_start(
            xbf[ct * P:(ct + 1) * P, :], x[ct * P:(ct + 1) * P, :])
    for mt in range(MT):
        # [C, 128] column panel of Xbf -> transposed [128, C] into xT
        nc.sync.dma_start_transpose(
            xT[:, mt * C:(mt + 1) * C],
            xbf[:, mt * P:(mt + 1) * P])


def _build_nc():
    nc = bacc.Bacc(
        "TRN2",
        target_bir_lowering=False,
        debug=False,
        num_devices=E,
    )
    x = nc.declare_dram_parameter("dispatched_input", [C, M], F32, isOutput=False)
    w1 = nc.declare_dram_parameter("inner_experts", [M, H], F32, isOutput=False)
    w2 = nc.declare_dram_parameter("out_experts", [H, M], F32, isOutput=False)
    y = nc.declare_dram_parameter("out", [C, M], F32, isOutput=True)

    CS = 1024         # phase-2 c-strip cached in SBUF
    NCS = C // CS     # 2
    MC = 512          # phase-2 m chunk (one PSUM bank)
    NMC = M // MC     # 4
    SUB = 8           # h-tiles per aT subtile
    NSUB = HT // SUB  # 8 subtiles per strip
    N_A = 5           # subtiles in the dedicated (cross-phase) zone

    with tile.TileContext(nc) as tc:
        with tc.tile_pool(name="dram", bufs=1, space="DRAM") as dram_pool:
            # actT spill: one DRAM tile per 128-row h-block so phase-2
            # prefetch depends only on the rows it reads.
            actT = [dram_pool.tile([P, C], BF, name=f"actT_{ht}", tag=f"actT_{ht}")
                    for ht in range(HT)]

            with tc.tile_pool(name="aTa", bufs=N_A) as aTa_pool:
                with tc.tile_pool(name="xT", bufs=1) as xT_pool:
                    xT = xT_pool.tile([P, MT * C], BF)

                    # ---- Phase 0 ----
                    if PHASE0_MODE == "xbar":
                        _phase0_xbar(nc, tc, x, xT, dram_pool)
                    else:
                        _phase0_pe(nc, tc, x, xT)

                    # ---- Phase 1: actT = relu(W1.T @ X.T), stream W1 once ----
                    HS = 512          # h panel width staged at a time
                    NHS = H // HS     # 16
                    with tc.tile_pool(name="w1s", bufs=3) as w1s_pool, \
                         tc.tile_pool(name="w1b", bufs=24) as w1b_pool, \
                         tc.tile_pool(name="ps1", bufs=8, space="PSUM") as ps1_pool, \
                         tc.tile_pool(name="acts", bufs=3) as act_pool:
                        for hs in range(NHS):
                            w1b_tiles = []
                            for mt in range(MT):
                                ws = w1s_pool.tile([P, HS], F32, tag="w1s",
                                                   name=f"w1s{hs}_{mt}")
                                nc.sync.dma_start(
                                    ws[:],
                                    w1[mt * P:(mt + 1) * P, hs * HS:(hs + 1) * HS])
                                wb = w1b_pool.tile([P, HS], BF, tag="w1b",
                                                   name=f"w1b{hs}_{mt}")
                                nc.vector.tensor_copy(wb[:], ws[:])
                                w1b_tiles.append(wb)
                            for hb in range(HS // P):  # 4 h-blocks of 128
                                pss = [ps1_pool.tile([P, FD], F32, tag="ps1",
                                                     name=f"ps1_{hs}_{hb}_{i}")
                                       for i in range(C // FD)]
                                for mt in range(MT):
                                    lhsT = w1b_tiles[mt][:, hb * P:(hb + 1) * P]
                                    for cc in range(C // FD):
                                        nc.tensor.matmul(
                                            pss[cc][:],
                                            lhsT,
                                            xT[:, mt * C + cc * FD: mt * C + (cc + 1) * FD],
                                            start=(mt == 0),
                                            stop=(mt == MT - 1),
                                        )
                                at = act_pool.tile([P, C], BF, tag="acts",
                                                   name=f"acts{hs}_{hb}")
                                for cc in range(C // FD):
                                    nc.scalar.activation(
                                        at[:, cc * FD:(cc + 1) * FD],
                                        pss[cc][:],
                                        mybir.ActivationFunctionType.Relu,
                                    )
                                ht = hs * (HS // P) + hb
                                nc.sync.dma_start(actT[ht][:], at[:])

                # ---- Phase 2: Y = actT.T @ W2, c-strips cached ----
                # (xT pool closed; its zone is recycled by the pools below.
                # aTa keeps a dedicated zone so strip-0 loads overlap phase 1.)
                with tc.tile_pool(name="aTb", bufs=NSUB - N_A) as aTb_pool, \
                     tc.tile_pool(name="w2s", bufs=4) as w2s_pool, \
                     tc.tile_pool(name="w2b", bufs=4) as w2b_pool, \
                     tc.tile_pool(name="ps2", bufs=8, space="PSUM") as ps2_pool, \
                     tc.tile_pool(name="ostage", bufs=8) as o_pool:
                    for cs in range(NCS):
                        # load strip: 8 subtiles of [128, SUB*CS] bf16
                        subs = []
                        for k in range(NSUB):
                            pool = aTa_pool if k < N_A else aTb_pool
                            sub = pool.tile([P, SUB * CS], BF,
                                            tag=f"aT{'a' if k < N_A else 'b'}",
                                            name=f"aT_{cs}_{k}")
                            for j in range(SUB):
                                ht = k * SUB + j
                                # SWDGE (Pool) queue: issues in parallel with
                                # phase-1's SP-queue DMAs, so strip loads
                                # overlap the phase-1 tail instead of queuing
                                # behind it.
                                nc.gpsimd.dma_start(
                                    sub[:, j * CS:(j + 1) * CS],
                                    actT[ht][:, cs * CS:(cs + 1) * CS])
                            subs.append(sub)
                        for mc in range(NMC):
                            pcs = [ps2_pool.tile([P, MC], F32, tag="ps2",
                                                 name=f"ps2_{cs}_{mc}_{i}")
                                   for i in range(CS // P)]
                            for ht in range(HT):
                                ws = w2s_pool.tile([P, MC], F32, tag="w2s",
                                                   name=f"w2s{cs}_{mc}_{ht}")
                                nc.sync.dma_start(
                                    ws[:],
                                    w2[ht * P:(ht + 1) * P, mc * MC:(mc + 1) * MC])
                                wb = w2b_pool.tile([P, MC], BF, tag="w2b",
                                                   name=f"w2b{cs}_{mc}_{ht}")
                                nc.vector.tensor_copy(wb[:], ws[:])
                                sub = subs[ht // SUB]
                                off = (ht % SUB) * CS
                                for ct in range(CS // P):
                                    nc.tensor.matmul(
                                        pcs[ct][:],
                                        sub[:, off + ct * P: off + (ct + 1) * P],
                                        wb[:],
                                        start=(ht == 0),
                                        stop=(ht == HT - 1),
                                    )
                            for ct in range(CS // P):
                                ob = o_pool.tile([P, MC], F32, tag="ostage",
                                                 name=f"ob{cs}_{mc}_{ct}")
                                nc.vector.tensor_copy(ob[:], pcs[ct][:])
                                c0 = cs * CS + ct * P
                                nc.sync.dma_start(
                                    y[c0:c0 + P, mc * MC:(mc + 1) * MC], ob[:])
    nc.compile()
    return nc


def get_nc():
    global _CACHED_NC
    if _CACHED_NC is None:
        _CACHED_NC = _build_nc()
    return _CACHED_NC


def kernel(dispatched_input, inner_experts, out_experts):
    dispatched_input = np.ascontiguousarray(dispatched_input, dtype=np.float32)
    inner_experts = np.ascontiguousarray(inner_experts, dtype=np.float32)
    out_experts = np.ascontiguousarray(out_experts, dtype=np.float32)
    assert dispatched_input.shape == (E, C, M)
    assert inner_experts.shape == (E, M, H)
    assert out_experts.shape == (E, H, M)

    nc = get_nc()
    in_maps = [
        {
            "dispatched_input": dispatched_input[e],
            "inner_experts": inner_experts[e],
            "out_experts": out_experts[e],
        }
        for e in range(E)
    ]
    res = run_bass_kernel_spmd(nc, in_maps, core_ids=list(range(E)))
    return np.stack([res.results[e]["out"] for e in range(E)], axis=0)


# revision 10
# speedup vs baseline: 38.3074x; 38.3074x over previous
"""MoE expert-parallel FFN kernel for Trainium2 (8 NeuronCores).

Problem: per-expert GEMM -> ReLU -> per-expert GEMM
  dispatched_input: (E=8, C=2048, M=2048) f32
  inner_experts:    (E=8, M=2048, H=8192) f32
  out_experts:      (E=8, H=8192, M=2048) f32
  out:              (E=8, C=2048, M=2048) f32

Sharding: pure expert parallelism — expert e runs entirely on core e.
No collectives needed.

Per-core dataflow (bf16 compute, fp32 PSUM accumulation):
  Phase 0: transpose X (C,M) -> X^T (M,C) resident in SBUF as bf16.
           Two modes: "xbar" (cast-DMA to DRAM then DMA-transpose loads,
           zero PE time) or "pe" (TensorE 128x128 transposes).
  Phase 1: actT[h,c] = relu(W1^T X^T) streamed over h, accumulated over
           m in PSUM. W1 streamed once; ReLU+cast on ScalarE; actT
           spilled to DRAM scratch as bf16 (32MB, one tile per 128-row
           h-block so phase 2 can prefetch rows as they land).
  Phase 2: Y[c,m] = actT^T @ W2 accumulated over h in PSUM. actT
           c-strips cached in SBUF (16MB as 8 subtiles; 5 live in a
           dedicated zone and loaded via the SWDGE queue so strip loads
           overlap the phase-1 tail); W2 streamed twice.

TimelineSim predicted per-core exec: ~1.82 ms (bf16 PE roofline 1.75 ms).
"""

import numpy as np

import concourse.bass as bass
import concourse.tile as tile
from concourse import bacc, mybir
from concourse.bass_utils import run_bass_kernel_spmd
from concourse.masks import make_identity

E = 8
C = 2048  # tokens per expert
M = 2048  # model dim
H = 8192  # ffn dim
P = 128   # partitions
FD = 512  # matmul moving free dim (one PSUM bank of fp32)

BF = mybir.dt.bfloat16
F32 = mybir.dt.float32

MT = M // P   # 16 m-tiles
CT = C // P   # 16 c-tiles
HT = H // P   # 64 h-tiles

CS = 1024         # phase-2 c-strip cached in SBUF
NCS = C // CS     # 2
MC = 512          # phase-2 m chunk (one PSUM bank)
NMC = M // MC     # 4
SUB = 8           # h-tiles per aT subtile
NSUB = HT // SUB  # 8 subtiles per strip
N_A = 5           # subtiles in the dedicated (cross-phase) zone

PHASE0_MODE = "xbar"   # "xbar" or "pe"

_CACHED = {}


def _phase0_pe(nc, tc, x, xT, rep):
    """TensorE-transpose path: X -> SBUF fp32 -> bf16 -> PE transpose."""
    with tc.tile_pool(name="xstage", bufs=3) as xs_pool, \
         tc.tile_pool(name="xbf", bufs=3) as xb_pool, \
         tc.tile_pool(name="tpsum", bufs=8, space="PSUM") as tp_pool, \
         tc.tile_pool(name="ident", bufs=1) as id_pool:
        ident = id_pool.tile([P, P], BF, name=f"ident{rep}")
        make_identity(nc, ident)
        for ct in range(CT):
            xs = xs_pool.tile([P, M], F32, tag="xs", name=f"xs{rep}_{ct}")
            for half in range(2):
                nc.sync.dma_start(
                    xs[:, half * (M // 2):(half + 1) * (M // 2)],
                    x[ct * P:(ct + 1) * P,
                      half * (M // 2):(half + 1) * (M // 2)])
            xb = xb_pool.tile([P, M], BF, tag="xb", name=f"xb{rep}_{ct}")
            for q in range(4):
                nc.vector.tensor_copy(
                    xb[:, q * FD:(q + 1) * FD], xs[:, q * FD:(q + 1) * FD])
                for mt in range(q * 4, (q + 1) * 4):
                    tp = tp_pool.tile([P, P], BF, tag="tp",
                                      name=f"tp{rep}_{ct}_{mt}")
                    nc.tensor.transpose(
                        tp[:], xb[:, mt * P:(mt + 1) * P], ident[:])
                    nc.vector.tensor_copy(
                        xT[:, mt * C + ct * P: mt * C + (ct + 1) * P], tp[:])


def _phase0_xbar(nc, tc, x, xT, dram_pool, rep):
    """Cast-DMA X to bf16 in DRAM, then xbar DMA-transpose into SBUF."""
    xbf = dram_pool.tile([C, M], BF, name=f"xbf{rep}", tag="xbf")
    for ct in range(CT):
        # SWDGE cast-DMA (fp32 -> bf16), DRAM -> DRAM
        nc.gpsimd.dma_start(
            xbf[ct * P:(ct + 1) * P, :], x[ct * P:(ct + 1) * P, :])
    for mt in range(MT):
        # [C, 128] column panel of Xbf -> transposed [128, C] into xT
        nc.sync.dma_start_transpose(
            xT[:, mt * C:(mt + 1) * C],
            xbf[:, mt * P:(mt + 1) * P])


def _phase1(nc, tc, x, w1, xT, actT, dram_pool, rep):
    """actT = relu(W1.T @ X.T); stream W1 once; spill actT to DRAM bf16."""
    HS = 512          # h panel width staged at a time
    NHS = H // HS     # 16
    with tc.tile_pool(name="w1s", bufs=3) as w1s_pool, \
         tc.tile_pool(name="w1b", bufs=24) as w1b_pool, \
         tc.tile_pool(name="ps1", bufs=8, space="PSUM") as ps1_pool, \
         tc.tile_pool(name="acts", bufs=3) as act_pool:
        for hs in range(NHS):
            w1b_tiles = []
            for mt in range(MT):
                ws = w1s_pool.tile([P, HS], F32, tag="w1s",
                                   name=f"w1s{rep}_{hs}_{mt}")
                nc.sync.dma_start(
                    ws[:], w1[mt * P:(mt + 1) * P, hs * HS:(hs + 1) * HS])
                wb = w1b_pool.tile([P, HS], BF, tag="w1b",
                                   name=f"w1b{rep}_{hs}_{mt}")
                nc.vector.tensor_copy(wb[:], ws[:])
                w1b_tiles.append(wb)
            for hb in range(HS // P):  # 4 h-blocks of 128
                pss = [ps1_pool.tile([P, FD], F32, tag="ps1",
                                     name=f"ps1_{rep}_{hs}_{hb}_{i}")
                       for i in range(C // FD)]
                for mt in range(MT):
                    lhsT = w1b_tiles[mt][:, hb * P:(hb + 1) * P]
                    for cc in range(C // FD):
                        nc.tensor.matmul(
                            pss[cc][:],
                            lhsT,
                            xT[:, mt * C + cc * FD: mt * C + (cc + 1) * FD],
                            start=(mt == 0),
                            stop=(mt == MT - 1),
                        )
                at = act_pool.tile([P, C], BF, tag="acts",
                                   name=f"acts{rep}_{hs}_{hb}")
                for cc in range(C // FD):
                    nc.scalar.activation(
                        at[:, cc * FD:(cc + 1) * FD],
                        pss[cc][:],
                        mybir.ActivationFunctionType.Relu,
                    )
                ht = hs * (HS // P) + hb
                nc.sync.dma_start(actT[ht][:], at[:])


def _phase2(nc, tc, w2, y, actT, aTa_pool, rep):
    """Y = actT.T @ W2, c-strips cached in SBUF, W2 streamed per strip."""
    with tc.tile_pool(name="aTb", bufs=NSUB - N_A) as aTb_pool, \
         tc.tile_pool(name="w2s", bufs=4) as w2s_pool, \
         tc.tile_pool(name="w2b", bufs=4) as w2b_pool, \
         tc.tile_pool(name="ps2", bufs=8, space="PSUM") as ps2_pool, \
         tc.tile_pool(name="ostage", bufs=8) as o_pool:
        for cs in range(NCS):
            subs = []
            for k in range(NSUB):
                pool = aTa_pool if k < N_A else aTb_pool
                sub = pool.tile([P, SUB * CS], BF,
                                tag=f"aT{'a' if k < N_A else 'b'}",
                                name=f"aT_{rep}_{cs}_{k}")
                for j in range(SUB):
                    ht = k * SUB + j
                    # SWDGE (Pool) queue: issues in parallel with phase-1's
                    # SP-queue DMAs, so strip loads overlap the phase-1 tail
                    # instead of queuing behind it.
                    nc.gpsimd.dma_start(
                        sub[:, j * CS:(j + 1) * CS],
                        actT[ht][:, cs * CS:(cs + 1) * CS])
                subs.append(sub)
            for mc in range(NMC):
                pcs = [ps2_pool.tile([P, MC], F32, tag="ps2",
                                     name=f"ps2_{rep}_{cs}_{mc}_{i}")
                       for i in range(CS // P)]
                for ht in range(HT):
                    ws = w2s_pool.tile([P, MC], F32, tag="w2s",
                                       name=f"w2s{rep}_{cs}_{mc}_{ht}")
                    nc.sync.dma_start(
                        ws[:], w2[ht * P:(ht + 1) * P, mc * MC:(mc + 1) * MC])
                    wb = w2b_pool.tile([P, MC], BF, tag="w2b",
                                       name=f"w2b{rep}_{cs}_{mc}_{ht}")
                    nc.vector.tensor_copy(wb[:], ws[:])
                    sub = subs[ht // SUB]
                    off = (ht % SUB) * CS
                    for ct in range(CS // P):
                        nc.tensor.matmul(
                            pcs[ct][:],
                            sub[:, off + ct * P: off + (ct + 1) * P],
                            wb[:],
                            start=(ht == 0),
                            stop=(ht == HT - 1),
                        )
                for ct in range(CS // P):
                    ob = o_pool.tile([P, MC], F32, tag="ostage",
                                     name=f"ob{rep}_{cs}_{mc}_{ct}")
                    nc.vector.tensor_copy(ob[:], pcs[ct][:])
                    c0 = cs * CS + ct * P
                    nc.sync.dma_start(
                        y[c0:c0 + P, mc * MC:(mc + 1) * MC], ob[:])


def _build_nc(repeats=1):
    nc = bacc.Bacc(
        "TRN2",
        target_bir_lowering=False,
        debug=False,
        num_devices=E,
    )
    x = nc.declare_dram_parameter("dispatched_input", [C, M], F32, isOutput=False)
    w1 = nc.declare_dram_parameter("inner_experts", [M, H], F32, isOutput=False)
    w2 = nc.declare_dram_parameter("out_experts", [H, M], F32, isOutput=False)
    y = nc.declare_dram_parameter("out", [C, M], F32, isOutput=True)

    with tile.TileContext(nc) as tc:
        with tc.tile_pool(name="dram", bufs=1, space="DRAM") as dram_pool:
            # actT spill: one DRAM tile per 128-row h-block so phase-2
            # prefetch depends only on the rows it reads.
            actT = [dram_pool.tile([P, C], BF, name=f"actT_{ht}",
                                   tag=f"actT_{ht}")
                    for ht in range(HT)]
            for rep in range(repeats):
                with tc.tile_pool(name="aTa", bufs=N_A) as aTa_pool:
                    with tc.tile_pool(name="xT", bufs=1) as xT_pool:
                        xT = xT_pool.tile([P, MT * C], BF, name=f"xT{rep}")
                        if PHASE0_MODE == "xbar":
                            _phase0_xbar(nc, tc, x, xT, dram_pool, rep)
                        else:
                            _phase0_pe(nc, tc, x, xT, rep)
                        _phase1(nc, tc, x, w1, xT, actT, dram_pool, rep)
                    _phase2(nc, tc, w2, y, actT, aTa_pool, rep)
    nc.compile()
    return nc


def get_nc(repeats=1):
    if repeats not in _CACHED:
        _CACHED[repeats] = _build_nc(repeats)
    return _CACHED[repeats]


def kernel(dispatched_input, inner_experts, out_experts):
    dispatched_input = np.ascontiguousarray(dispatched_input, dtype=np.float32)
    inner_experts = np.ascontiguousarray(inner_experts, dtype=np.float32)
    out_experts = np.ascontiguousarray(out_experts, dtype=np.float32)
    assert dispatched_input.shape == (E, C, M)
    assert inner_experts.shape == (E, M, H)
    assert out_experts.shape == (E, H, M)

    nc = get_nc()
    in_maps = [
        {
            "dispatched_input": dispatched_input[e],
            "inner_experts": inner_experts[e],
            "out_experts": out_experts[e],
        }
        for e in range(E)
    ]
    res = run_bass_kernel_spmd(nc, in_maps, core_ids=list(range(E)))
    return np.stack([res.results[e]["out"] for e in range(E)], axis=0)
